# revision 28
# baseline (speedup 1.0000x reference)
"""Gated-relative-position-bias multi-head attention, 8-way tensor-parallel
over heads on Trainium2 (Bass/Tile).

Shapes: x (2, 2048, 1024), 16 heads x 64 head-dim, position_bias
(16, 2048, 2048), per-query sigmoid gates computed from x.

Sharding: core c owns heads (2c, 2c+1) = feature slice [128c, 128c+128).
Each core computes q/k/v for its heads, the gated-bias attention, and a
partial output projection (O_g @ Wo_g.T).  The host sums the 8 partials and
adds bo.

Per-core strategy:
  - host passes xT (x transposed, [D, B*T]) so the d-contraction sits on
    partitions; qT/kT/vT are computed weights-stationary (1/sqrt(hd) folded
    into Wq/bq on the host).
  - the gate pipeline (small matmul + sigmoid + combine + partition
    broadcast) runs FIRST so the attention phase is never gated on it.
  - scores are computed TRANSPOSED: sT[k, q] = kT.T @ qT (K=hd=64), so the
    attn @ v contraction (over k) has k on partitions.
  - the gated position bias is added into the scores PSUM by the PE as two
    concurrent row+col-tiled K=64 identity matmuls (disjoint partition
    halves of the same bank), overlapping each other in the array.
  - softmax needs no max-subtraction (scores are O(+-3) for this model
    family); denominators come free as an all-ones column of v_aug (row 64
    of the AV matmul PSUM output).
  - batch 0's output projection is software-pipelined into batch 1's first
    attention block; bf16 stores alternate between the two HWDGE queues so
    they never block pbt loads.
"""

import sys

sys.path.insert(0, "/opt/trn_rl_repo")

import ml_dtypes
import numpy as np

import concourse.mybir as mybir
import concourse.tile as tile
from concourse import bacc
from concourse import bass_utils
from concourse.bass_utils import run_bass_kernel_spmd

# (walrus's LDW optimization rejects this kernel's transpose-mode weight
# loads — "InstLdweights is not compatible with LDW optimization" — so the
# default --enable-ldw-opt=false stays.)

F32 = mybir.dt.float32
BF16 = mybir.dt.bfloat16
AF = mybir.ActivationFunctionType
ALU = mybir.AluOpType

B, T, D, H, HD = 2, 2048, 1024, 16, 64
NCORES = 8
HPC = H // NCORES          # heads per core = 2
FPC = HPC * HD             # features per core = 128
BT = B * T                 # 4096
P = 128
NKC = T // P               # key chunks = 16
NQQ = T // 512             # q quarters = 4
VW = HD + 2                # v_aug chunk width (64 v cols + ones + pad)

# test.py hooks
TRACE = False
LAST_RESULT = None


def _build_program():
    nc = bacc.Bacc("TRN2", target_bir_lowering=False, debug=False,
                   num_devices=NCORES)

    xT = nc.dram_tensor("xT", [D, BT], BF16, kind="ExternalInput")
    xg = nc.dram_tensor("xg", [P, BT], BF16, kind="ExternalInput")
    wq = nc.dram_tensor("wq", [D, FPC], BF16, kind="ExternalInput")
    wk = nc.dram_tensor("wk", [D, FPC], BF16, kind="ExternalInput")
    wv = nc.dram_tensor("wv", [D, FPC], BF16, kind="ExternalInput")
    bq = nc.dram_tensor("bq", [FPC], F32, kind="ExternalInput")
    bk = nc.dram_tensor("bk", [FPC], F32, kind="ExternalInput")
    bv = nc.dram_tensor("bv", [FPC], F32, kind="ExternalInput")
    wo = nc.dram_tensor("wo", [FPC, D], BF16, kind="ExternalInput")
    pbt = nc.dram_tensor("pbt", [HPC, T, T], BF16, kind="ExternalInput")
    wg2 = nc.dram_tensor("wg2", [P, 128], BF16, kind="ExternalInput")
    bg2 = nc.dram_tensor("bg2", [97], F32, kind="ExternalInput")
    gc2 = nc.dram_tensor("gc2", [97], F32, kind="ExternalInput")
    idb = nc.dram_tensor("idb", [P, P], BF16, kind="ExternalInput")
    out = nc.dram_tensor("out", [BT, D], BF16, kind="ExternalOutput")

    with tile.TileContext(nc) as tc, \
         tc.tile_pool(name="const", bufs=1) as const, \
         tc.tile_pool(name="big", bufs=1) as big, \
         tc.tile_pool(name="xt", bufs=2) as xt_pool, \
         tc.tile_pool(name="xgp", bufs=2) as xg_pool, \
         tc.tile_pool(name="vt", bufs=1) as vt_pool, \
         tc.tile_pool(name="gp", bufs=1) as g_pool, \
         tc.tile_pool(name="pb", bufs=6) as pb_pool, \
         tc.tile_pool(name="pbg", bufs=6) as pbg_pool, \
         tc.tile_pool(name="expp", bufs=8) as exp_pool, \
         tc.tile_pool(name="gbc", bufs=1) as gbc_pool, \
         tc.tile_pool(name="rbp", bufs=4) as rb_pool, \
         tc.tile_pool(name="osb", bufs=3) as osb_pool, \
         tc.tile_pool(name="ps", bufs=4, space="PSUM") as ps:
        # ---------------- gate-path constants first ----------------
        ones_t = const.tile([P, HD], F32, tag="ones")
        nc.vector.memset(ones_t[:], 1.0)
        wg2_t = const.tile([P, 128], BF16, tag="wg2")
        nc.sync.dma_start(wg2_t[:], wg2[:])
        bg2_t = const.tile([97, 1], F32, tag="bg2")
        nc.sync.dma_start(bg2_t[:], bg2.rearrange("(p o) -> p o", o=1))
        gc_t = const.tile([97, 1], F32, tag="gc")
        nc.sync.dma_start(gc_t[:], gc2.rearrange("(p o) -> p o", o=1))

        qT = big.tile([FPC, BT], BF16, tag="qT")
        kT = big.tile([FPC, BT], BF16, tag="kT")
        # G rows: 0/32 = gate-a (h0/h1), 64/96 = gate-b
        G = g_pool.tile([97, BT], BF16, tag="G")
        # gt1 rows 0/32: scratch, then the combined per-head gate G2
        gt1 = g_pool.tile([33, BT], BF16, tag="gt1")
        OT = [big.tile([FPC, T], BF16, tag=f"OT{b}", name=f"OT{b}")
              for b in range(B)]
        vaug = {(h, b): big.tile([P, NKC * VW], BF16,
                                 tag=f"va{h}{b}", name=f"va{h}{b}")
                for h in range(HPC) for b in range(B)}

        # PE warm-up: ~4us of dummy matmuls at t=0 fill the initial DMA wait
        # and push the HAM clock-gate to 8/8 before real work arrives.
        def warm_burst(n, tag):
            w = ps.tile([HD, HD], F32, tag="sc", name=f"warm{tag}")
            for _ in range(n):
                nc.tensor.matmul(w[:], ones_t[:, 0:HD], ones_t[:, 0:HD],
                                 start=True, stop=True)

        warm_burst(20, "a")
        # ---------------- gate pipeline (first: off the critical path) ----
        for c8 in range(BT // 512):
            c0 = c8 * 512
            xg_t = xg_pool.tile([P, 512], BF16, tag="xgc")
            nc.sync.dma_start(xg_t[:], xg[:, c0:c0 + 512])
            psg = ps.tile([P, 512], F32, tag="sc", name=f"psg{c8}")
            nc.tensor.matmul(psg[:], wg2_t[:], xg_t[:], start=True, stop=True)
            nc.scalar.activation(G[:, c0:c0 + 512], psg[0:97, :], AF.Sigmoid,
                                 bias=bg2_t[:])

        warm_burst(14, "b")
        # gate combine per head: G2 = a * (b * gc - 1) + 2
        for h in range(HPC):
            r = 32 * h
            nc.vector.tensor_scalar(
                out=gt1[r:r + 1, :], in0=G[64 + r:65 + r, :],
                scalar1=gc_t[64 + r:65 + r, :], scalar2=-1.0,
                op0=ALU.mult, op1=ALU.add)
            nc.vector.tensor_mul(gt1[r:r + 1, :], G[r:r + 1, :],
                                 gt1[r:r + 1, :])
            nc.vector.tensor_scalar(out=gt1[r:r + 1, :], in0=gt1[r:r + 1, :],
                                    scalar1=2.0, scalar2=None, op0=ALU.add)

        # gate broadcasts (gpsimd, overlaps the qkv phase)
        gbcs = {}
        for h in range(HPC):
            for b in range(B):
                gbc = gbc_pool.tile([P, T], BF16, tag=f"gbc{h}{b}",
                                    name=f"gbc{h}{b}")
                nc.gpsimd.partition_broadcast(
                    gbc[:], gt1[32 * h:32 * h + 1, b * T:(b + 1) * T])
                gbcs[(h, b)] = gbc

        # ---------------- q/k/v projection weights ----------------
        w_ts = {}
        for name, dram in (("wq", wq), ("wk", wk), ("wv", wv)):
            w_t = const.tile([P, D // P, FPC], BF16, tag=name, name=name + "w")
            nc.sync.dma_start(w_t[:], dram.rearrange("(c p) f -> p c f", p=P))
            w_ts[name] = w_t
        b_ts = {}
        for name, dram in (("bq", bq), ("bk", bk), ("bv", bv)):
            b_t = const.tile([FPC, 1], F32, tag=name, name=name + "b")
            nc.sync.dma_start(b_t[:], dram.rearrange("(p o) -> p o", o=1))
            b_ts[name] = b_t
        idb_t = const.tile([P, P], BF16, tag="idb")
        nc.sync.dma_start(idb_t[:], idb[:])

        # ---------------- q/k/v projections ----------------
        warm_burst(32, "c")
        vT = vt_pool.tile([FPC, BT], BF16, tag="vT")
        for th in range(4):
            tsl = slice(th * (BT // 4), (th + 1) * (BT // 4))
            xts = []
            for d in range(D // P):
                xt_t = xt_pool.tile([P, BT // 4], BF16, tag=f"xt{d}",
                                    name=f"xt{d}_{th}")
                nc.sync.dma_start(xt_t[:], xT[d * P:(d + 1) * P, tsl])
                xts.append(xt_t)
            for ti in range(BT // 4 // 512):
                c0 = th * (BT // 4) + ti * 512
                for wname, bname, dst in (("wq", "bq", qT), ("wk", "bk", kT),
                                          ("wv", "bv", vT)):
                    psq = ps.tile([FPC, 512], F32, tag="sc",
                                  name=f"psq{wname}{th}{ti}")
                    for d in range(D // P):
                        nc.tensor.matmul(
                            psq[:], w_ts[wname][:, d, :],
                            xts[d][:, ti * 512:(ti + 1) * 512],
                            start=(d == 0), stop=(d == D // P - 1))
                    nc.vector.tensor_scalar(
                        out=dst[:, c0:c0 + 512], in0=psq[:],
                        scalar1=b_ts[bname][:], scalar2=None, op0=ALU.add)

        wo_t = const.tile([FPC, D], BF16, tag="wo")
        nc.sync.dma_start(wo_t[:], wo[:])

        # v_aug: both heads transposed per 128-token chunk + ones columns
        for b in range(B):
            for h in range(HPC):
                nc.vector.memset(vaug[(h, b)][:], 1.0)
        for b in range(B):
            for kc in range(NKC):
                pst = ps.tile([P, P], BF16, tag="av", name=f"pst{b}{kc}")
                nc.tensor.transpose(
                    pst[:], vT[:, b * T + kc * P: b * T + (kc + 1) * P],
                    idb_t[:])
                for h in range(HPC):
                    nc.vector.tensor_copy(
                        vaug[(h, b)][:, kc * VW:kc * VW + HD],
                        pst[:, h * HD:(h + 1) * HD])

        # ---------------- attention ----------------
        def out_proj_tile(b, tt, tail):
            # one 128-token chunk of the output projection for batch b
            ob = osb_pool.tile([P, D], BF16, tag="ob", name=f"ob{b}{tt}")
            for s in range(D // 512):
                po = ps.tile([P, 512], F32, tag="sc", name=f"po{b}{tt}{s}")
                nc.tensor.matmul(po[:], OT[b][:, tt * P:(tt + 1) * P],
                                 wo_t[:, s * 512:(s + 1) * 512],
                                 start=True, stop=True)
                if tail and s == 1:
                    nc.scalar.copy(ob[:, 512:1024], po[:])
                else:
                    nc.vector.tensor_copy(ob[:, s * 512:(s + 1) * 512], po[:])
            if tail:
                deng = nc.sync if tt % 2 == 0 else nc.scalar
            else:
                deng = nc.sync
            deng.dma_start(out[b * T + tt * P: b * T + (tt + 1) * P, :],
                           ob[:])

        def attn_block(h, b, epilogue=None):
            hsl = slice(h * HD, (h + 1) * HD)
            va = vaug[(h, b)]
            avs = [ps.tile([HD + 2, 512], F32, tag="av",
                           name=f"av{h}{b}{qq}") for qq in range(NQQ)]
            for kc in range(NKC):
                pbt_t = pb_pool.tile([P, T], BF16, tag="pb")
                nc.sync.dma_start(pbt_t[:], pbt[h, kc * P:(kc + 1) * P, :])
                pbg = pbg_pool.tile([P, T], BF16, tag="pbg")
                eng = nc.gpsimd if kc % 4 == 0 else nc.vector
                eng.tensor_tensor(out=pbg[:], in0=pbt_t[:],
                                  in1=gbcs[(h, b)][:], op=ALU.mult)
                lk = kT[hsl, b * T + kc * P: b * T + (kc + 1) * P]
                vak = va[:, kc * VW:(kc + 1) * VW]
                scs = []
                for qq in range(NQQ):
                    q0 = b * T + qq * 512
                    sc = ps.tile([P, 512], F32, tag="sc",
                                 name=f"sc{h}{b}{kc}{qq}")
                    nc.tensor.matmul(sc[:], lk, qT[hsl, q0:q0 + 512],
                                     start=True, stop=False)
                    scs.append(sc)
                # gated position bias added by the PE via identity matmul
                for qq in range(NQQ):
                    q0 = qq * 512
                    nc.tensor.matmul(scs[qq][:], idb_t[:],
                                     pbg[:, q0:q0 + 512],
                                     start=False, stop=True)
                exs = []
                for qq in range(NQQ):
                    ex = exp_pool.tile([P, 512], BF16, tag="ex")
                    nc.scalar.activation(ex[:], scs[qq][:], AF.Exp)
                    exs.append(ex)
                for qq in range(NQQ):
                    nc.tensor.matmul(avs[qq][:], vak, exs[qq][:],
                                     start=(kc == 0),
                                     stop=(kc == NKC - 1))
                if epilogue is not None:
                    epilogue(kc)
            # evacuate the AV accumulators to SBUF right away (frees the
            # PSUM banks for the next block), then normalize lazily:
            # denominators live in row HD; the final multiply runs on
            # GpSimd so it never blocks the next block's pbg/score path.
            av_sbs = []
            for qq in range(NQQ):
                av_sb = rb_pool.tile([HD + 1, 512], F32, tag="avsb",
                                     name=f"avsb{h}{b}{qq}")
                nc.vector.tensor_copy(av_sb[:], avs[qq][0:HD + 1, :])
                av_sbs.append(av_sb)
            for qq in range(NQQ):
                av_sb = av_sbs[qq]
                rbp = ps.tile([HD, 512], F32, tag="sc",
                              name=f"rbp{h}{b}{qq}")
                nc.tensor.matmul(rbp[:], ones_t[HD:HD + 1, :],
                                 av_sb[HD:HD + 1, :], start=True, stop=True)
                rbr = rb_pool.tile([HD, 512], F32, tag="rbr")
                nc.vector.reciprocal_approx_fast(rbr[:], rbp[:])
                nc.gpsimd.tensor_tensor(
                    out=OT[b][hsl, qq * 512:(qq + 1) * 512],
                    in0=av_sb[0:HD, :], in1=rbr[:], op=ALU.mult)

        attn_block(0, 0)
        attn_block(1, 0)
        # batch 0's output projection rides along batch 1's first block
        attn_block(0, 1, epilogue=lambda kc: out_proj_tile(0, kc, False))
        attn_block(1, 1)
        for tt in range(T // P):
            out_proj_tile(1, tt, True)

    nc.compile()
    return nc


_PROGRAM = None


def _get_program():
    global _PROGRAM
    if _PROGRAM is None:
        _PROGRAM = _build_program()
    return _PROGRAM


def kernel(x, position_bias, Wq, bq, Wk, bk, Wv, bv, Wo, bo, Wg, bg,
           gru_const):
    global LAST_RESULT
    x = np.asarray(x, dtype=np.float32)
    position_bias = np.asarray(position_bias, dtype=np.float32)
    Wq = np.asarray(Wq, dtype=np.float32)
    Wk = np.asarray(Wk, dtype=np.float32)
    Wv = np.asarray(Wv, dtype=np.float32)
    Wo = np.asarray(Wo, dtype=np.float32)
    bq = np.asarray(bq, dtype=np.float32)
    bk = np.asarray(bk, dtype=np.float32)
    bv = np.asarray(bv, dtype=np.float32)
    bo = np.asarray(bo, dtype=np.float32)
    Wg = np.asarray(Wg, dtype=np.float32)
    bg = np.asarray(bg, dtype=np.float32)
    gru_const = np.asarray(gru_const, dtype=np.float32)

    scale = np.float32(1.0 / np.sqrt(np.float32(HD)))

    xT_np = np.ascontiguousarray(x.reshape(BT, D).T)           # [D, BT]
    idb_np = np.eye(P).astype(ml_dtypes.bfloat16)
    # the reshape-(2,4)-sum of the 8 gate features is linear -> fold into
    # the weights:  Wg2[g] = sum of Wg rows [4g, 4g+4)
    Wg2 = Wg.reshape(2, 4, HD).sum(1)                          # [2, HD]
    bg2v = bg.reshape(2, 4).sum(1)                             # [2]

    in_maps = []
    for c in range(NCORES):
        fsl = slice(c * FPC, (c + 1) * FPC)
        wg2_np = np.zeros((P, 128), dtype=np.float32)
        bg2_np = np.zeros((97,), dtype=np.float32)
        # rows 0/32 = gate-a for head0/head1; rows 64/96 = gate-b
        wg2_np[0:HD, 0] = Wg2[0]
        wg2_np[HD:P, 32] = Wg2[0]
        wg2_np[0:HD, 64] = Wg2[1]
        wg2_np[HD:P, 96] = Wg2[1]
        bg2_np[[0, 32]] = bg2v[0]
        bg2_np[[64, 96]] = bg2v[1]
        gc2_np = np.zeros((97,), dtype=np.float32)
        gc2_np[64] = gru_const[0, c * HPC, 0, 0]
        gc2_np[96] = gru_const[0, c * HPC + 1, 0, 0]
        in_maps.append({
            "xT": xT_np.astype(ml_dtypes.bfloat16),
            "xg": np.ascontiguousarray(xT_np[fsl, :]).astype(ml_dtypes.bfloat16),
            "wq": (np.ascontiguousarray(Wq.T[:, fsl]) * scale).astype(ml_dtypes.bfloat16),
            "wk": np.ascontiguousarray(Wk.T[:, fsl]).astype(ml_dtypes.bfloat16),
            "wv": np.ascontiguousarray(Wv.T[:, fsl]).astype(ml_dtypes.bfloat16),
            "bq": np.ascontiguousarray(bq[fsl]) * scale,
            "bk": np.ascontiguousarray(bk[fsl]),
            "bv": np.ascontiguousarray(bv[fsl]),
            "wo": np.ascontiguousarray(Wo[:, fsl].T).astype(ml_dtypes.bfloat16),
            "pbt": np.ascontiguousarray(
                position_bias[c * HPC:(c + 1) * HPC].transpose(0, 2, 1)
            ).astype(ml_dtypes.bfloat16),
            "wg2": wg2_np.astype(ml_dtypes.bfloat16),
            "bg2": bg2_np,
            "gc2": gc2_np,
            "idb": idb_np,
        })

    nc = _get_program()
    res = run_bass_kernel_spmd(nc, in_maps, core_ids=list(range(NCORES)),
                               trace=TRACE)
    LAST_RESULT = res
    acc = res.results[0]["out"].astype(np.float32).copy()
    for c in range(1, NCORES):
        acc += res.results[c]["out"].astype(np.float32)
    acc += bo[None, :]
    return acc.reshape(B, T, D)
